# revision 27
# baseline (speedup 1.0000x reference)
"""NodeRoIPool Trainium2 kernel — spatial-sharded dense box filter.

For each of 20000 ROIs (8 corner coords), 5 points (4 edge midpoints +
centroid) are snapped to the feature grid (ceil, clip to [2,254]) and a
4x4 window of feat [256,256,256] is mean-pooled across all 256 channels,
giving out [20000, 1280] (point-major, channel-fastest).

The 4x4 mean only depends on the snapped point, so the kernel just
box-filters the feature map densely and the host does the per-point
lookup (a pure-index numpy gather; point snapping does not depend on
feat, so no device work is saved by doing it on device — per-point
gathers cost ~25 ns/point on every device path, dwarfing the dense
filter).

Sharding (8 cores): each core owns a 32-row y band of the feature map
(all 256 channels). Per core:
  - host sends the fp16 feat slice (36 rows incl. 2-row halos, zero
    padded at the map edges)
  - device box-filters its band in fp16 with DVE shift-adds (vertical
    4-tap first — fewer rows for the wide horizontal passes), leaving
    the unscaled filtered band in natural [channel, y, x] layout
  - the filtered band is DMA'd out densely as fp16, chunk by chunk,
    overlapping the filter of the next chunk
Host gathers the 100000 points from the 8 bands, applies the 1/16 mean
scale, and converts to f32.
"""

import numpy as np

import concourse.bass as bass
import concourse.tile as tile
from concourse import bacc, mybir
from concourse import bass_utils

N_CORES = 8
C, H, W = 256, 256, 256
N_ROIS = 20000
BAND = 32            # y rows owned per core
ROWS_IN = BAND + 4   # loaded rows incl. halo (y0-2 .. y0+33)
NCK = 2              # y chunks per core
CKY = BAND // NCK    # 16 out rows per chunk
F32 = mybir.dt.float32
F16 = mybir.dt.float16

_prog_cache = {}


def _build_program():
    nc = bacc.Bacc("TRN2", target_bir_lowering=False, debug=False,
                   num_devices=N_CORES)

    feat_in = nc.dram_tensor("feat", [C, ROWS_IN, W], F16, kind="ExternalInput")
    out_t = nc.dram_tensor("out", [2, 128, BAND, W], F16,
                           kind="ExternalOutput")

    with tile.TileContext(nc) as tc:
        with tc.tile_pool(name="sbuf", bufs=1) as pool:
            vbs = []
            fins = []
            for cb in range(2):
                vb = pool.tile([128, BAND, W], F16, tag=f"vb{cb}")
                vbs.append(vb)
                fin = pool.tile([128, ROWS_IN, W], F16, tag=f"fin{cb}")
                fins.append(fin)
            # loads across the two HWDGE rings (sync, scalar): the very
            # first chunk's cb0 rows split across BOTH rings so the first
            # filter op starts earliest; sync stays at 2 loads + 5 stores
            # (builds with 8+ DMAs on the sync ring ran all DVE ops ~20%
            # slower, reproducibly)
            load_plan = (
                (nc.sync, 0, 0, 10), (nc.scalar, 0, 10, CKY + 3),
                (nc.scalar, 1, 0, ROWS_IN - 1),
                (nc.sync, 0, CKY + 3, ROWS_IN - 1),
            )
            for eng, cb, ra, rb in load_plan:
                eng.dma_start(
                    out=fins[cb][:, ra:rb, :],
                    in_=feat_in[cb * 128:(cb + 1) * 128, ra:rb, :])

            # h1 lives at +2 columns in a front-padded tile so h2 can
            # write full contiguous 256-wide rows; the pad columns feed
            # only x 0,1,255 of vb, which are never looked up (zeroed
            # once here to keep them finite)
            h1 = pool.tile([128, CKY, W + 8], F16, tag="h1")
            nc.vector.memset(h1[:, :, 0:2], 0.0)
            nc.vector.memset(h1[:, :, W + 1:W + 8], 0.0)

            for ck in range(NCK):
                r0 = ck * CKY
                for cb in range(2):
                    fin = fins[cb]
                    # vertical 4-tap (windows [y-2, y+1]) then horizontal
                    v1 = pool.tile([128, CKY + 2, W], F16, tag="v1")
                    nc.vector.tensor_tensor(
                        out=v1[:], in0=fin[:, r0:r0 + CKY + 2, :],
                        in1=fin[:, r0 + 1:r0 + CKY + 3, :],
                        op=mybir.AluOpType.add)
                    v2 = pool.tile([128, CKY, W], F16, tag="v2")
                    nc.vector.tensor_tensor(
                        out=v2[:], in0=v1[:, 0:CKY, :],
                        in1=v1[:, 2:CKY + 2, :], op=mybir.AluOpType.add)
                    nc.vector.tensor_tensor(
                        out=h1[:, :, 2:W + 1], in0=v2[:, :, 0:W - 1],
                        in1=v2[:, :, 1:W], op=mybir.AluOpType.add)
                    # split the very last chunk's h2+store into two 8-row
                    # halves so the final store starts 1 op earlier
                    halves = ((0, CKY),) if (ck, cb) != (NCK - 1, 1) \
                        else ((0, CKY // 2), (CKY // 2, CKY))
                    for ha, hb in halves:
                        nc.vector.tensor_tensor(
                            out=vbs[cb][:, r0 + ha:r0 + hb, :],
                            in0=h1[:, ha:hb, 0:W],
                            in1=h1[:, ha:hb, 2:W + 2],
                            op=mybir.AluOpType.add)
                        nc.sync.dma_start(
                            out=out_t[cb, :, r0 + ha:r0 + hb, :],
                            in_=vbs[cb][:, r0 + ha:r0 + hb, :])

    nc.compile()
    return nc


def _point_indices(rois):
    """Replicate the reference's f32 point snapping on host."""
    rr = (rois * np.float32(0.25)).reshape(-1, 4, 2)
    mids = (rr + np.roll(rr, -1, axis=1)) * np.float32(0.5)
    center = (((rr[:, 0] + rr[:, 1]) + rr[:, 2]) + rr[:, 3])
    center = (center * np.float32(0.25))[:, None, :]
    pts = np.concatenate([mids, center], axis=1)          # [N, 5, 2]
    xc = np.clip(np.ceil(pts[..., 0]), 2, 254).astype(np.int64).ravel()
    yc = np.clip(np.ceil(pts[..., 1]), 2, 254).astype(np.int64).ravel()
    return xc, yc


def kernel(feat: np.ndarray, rois: np.ndarray) -> np.ndarray:
    feat = np.ascontiguousarray(np.asarray(feat, dtype=np.float32))
    rois = np.ascontiguousarray(np.asarray(rois, dtype=np.float32))
    assert feat.shape == (C, H, W) and rois.shape == (N_ROIS, 8)

    if "nc" not in _prog_cache:
        _prog_cache["nc"] = _build_program()
    nc = _prog_cache["nc"]

    f16 = feat.astype(np.float16)
    in_maps = []
    for co in range(N_CORES):
        y0 = co * BAND
        fs = np.zeros((C, ROWS_IN, W), np.float16)
        a, b = max(0, y0 - 2), min(H, y0 + BAND + 2)
        fs[:, a - (y0 - 2):b - (y0 - 2), :] = f16[:, a:b, :]
        in_maps.append({"feat": fs})

    res = bass_utils.run_bass_kernel_spmd(
        nc, in_maps, core_ids=list(range(N_CORES)))

    # box[(y, x), c]: row-major global pixel id -> 256 channels
    box = np.empty((H * W, C), np.float16)
    bv = box.reshape(N_CORES, BAND * W, C)
    for co in range(N_CORES):
        o = np.asarray(res.results[co]["out"])   # [2, 128, BAND, W] fp16
        # -> [(ly, x), (cb, cl)]
        bv[co] = np.transpose(o.reshape(2, 128, BAND * W),
                              (2, 0, 1)).reshape(BAND * W, C)

    xc, yc = _point_indices(rois)
    pooled = box[yc * W + xc].astype(np.float32) * np.float32(1.0 / 16.0)
    return pooled.reshape(N_ROIS, 5 * C)


# revision 29
# speedup vs baseline: 1.1944x; 1.1944x over previous
"""NodeRoIPool Trainium2 kernel — spatial-sharded dense box filter.

For each of 20000 ROIs (8 corner coords), 5 points (4 edge midpoints +
centroid) are snapped to the feature grid (ceil, clip to [2,254]) and a
4x4 window of feat [256,256,256] is mean-pooled across all 256 channels,
giving out [20000, 1280] (point-major, channel-fastest).

The 4x4 mean only depends on the snapped point, so the kernel just
box-filters the feature map densely and the host does the per-point
lookup (a pure-index numpy gather; point snapping does not depend on
feat, so no device work is saved by doing it on device — per-point
gathers cost ~25 ns/point on every device path, dwarfing the dense
filter).

Sharding (8 cores): each core owns a 32-row y band of the feature map
(all 256 channels). Per core:
  - host sends the fp16 feat slice (36 rows incl. 2-row halos, zero
    padded at the map edges)
  - device box-filters its band in fp16 with DVE shift-adds (vertical
    4-tap first — fewer rows for the wide horizontal passes), leaving
    the unscaled filtered band in natural [channel, y, x] layout
  - the filtered band is DMA'd out densely as fp16, chunk by chunk,
    overlapping the filter of the next chunk
Host gathers the 100000 points from the 8 bands, applies the 1/16 mean
scale, and converts to f32.
"""

import numpy as np

import concourse.bass as bass
import concourse.tile as tile
from concourse import bacc, mybir
from concourse import bass_utils

N_CORES = 8
C, H, W = 256, 256, 256
N_ROIS = 20000
BAND = 32            # y rows owned per core
ROWS_IN = BAND + 4   # loaded rows incl. halo (y0-2 .. y0+33)
NCK = 2              # y chunks per core
CKY = BAND // NCK    # 16 out rows per chunk
F32 = mybir.dt.float32
F16 = mybir.dt.float16

_prog_cache = {}


def _build_program():
    nc = bacc.Bacc("TRN2", target_bir_lowering=False, debug=False,
                   num_devices=N_CORES)

    feat_in = nc.dram_tensor("feat", [C, ROWS_IN, W], F16, kind="ExternalInput")
    out_t = nc.dram_tensor("out", [2, 128, BAND, W], F16,
                           kind="ExternalOutput")

    with tile.TileContext(nc) as tc:
        with tc.tile_pool(name="sbuf", bufs=1) as pool:
            vbs = []
            fins = []
            for cb in range(2):
                vb = pool.tile([128, BAND, W], F16, tag=f"vb{cb}")
                vbs.append(vb)
                fin = pool.tile([128, ROWS_IN, W], F16, tag=f"fin{cb}")
                fins.append(fin)
            # loads across the two HWDGE rings (sync, scalar): the very
            # first chunk's cb0 rows split across BOTH rings so the first
            # filter op starts earliest; sync stays at 2 loads + 5 stores
            # (builds with 8+ DMAs on the sync ring ran all DVE ops ~20%
            # slower, reproducibly)
            load_plan = (
                (nc.sync, 0, 0, 10), (nc.scalar, 0, 10, CKY + 3),
                (nc.scalar, 1, 0, CKY + 3),
                (nc.sync, 0, CKY + 3, ROWS_IN - 1),
                (nc.scalar, 1, CKY + 3, ROWS_IN - 1),
            )
            for eng, cb, ra, rb in load_plan:
                eng.dma_start(
                    out=fins[cb][:, ra:rb, :],
                    in_=feat_in[cb * 128:(cb + 1) * 128, ra:rb, :])

            # h1 lives at +2 columns in a front-padded tile so h2 can
            # write full contiguous 256-wide rows; the pad columns feed
            # only x 0,1,255 of vb, which are never looked up (zeroed
            # once here to keep them finite)
            h1 = pool.tile([128, CKY, W + 4], F16, tag="h1")
            nc.vector.memset(h1[:, :, 0:2], 0.0)
            nc.vector.memset(h1[:, :, W + 1:W + 4], 0.0)

            for ck in range(NCK):
                r0 = ck * CKY
                for cb in range(2):
                    fin = fins[cb]
                    # vertical 4-tap (windows [y-2, y+1]) then horizontal
                    v1 = pool.tile([128, CKY + 2, W], F16, tag="v1")
                    nc.vector.tensor_tensor(
                        out=v1[:], in0=fin[:, r0:r0 + CKY + 2, :],
                        in1=fin[:, r0 + 1:r0 + CKY + 3, :],
                        op=mybir.AluOpType.add)
                    v2 = pool.tile([128, CKY, W], F16, tag="v2")
                    nc.vector.tensor_tensor(
                        out=v2[:], in0=v1[:, 0:CKY, :],
                        in1=v1[:, 2:CKY + 2, :], op=mybir.AluOpType.add)
                    nc.vector.tensor_tensor(
                        out=h1[:, :, 2:W + 1], in0=v2[:, :, 0:W - 1],
                        in1=v2[:, :, 1:W], op=mybir.AluOpType.add)
                    # split the very last chunk's h2+store into two 8-row
                    # halves so the final store starts 1 op earlier
                    halves = ((0, CKY),) if (ck, cb) != (NCK - 1, 1) \
                        else ((0, CKY // 2), (CKY // 2, CKY))
                    for ha, hb in halves:
                        nc.vector.tensor_tensor(
                            out=vbs[cb][:, r0 + ha:r0 + hb, :],
                            in0=h1[:, ha:hb, 0:W],
                            in1=h1[:, ha:hb, 2:W + 2],
                            op=mybir.AluOpType.add)
                        nc.sync.dma_start(
                            out=out_t[cb, :, r0 + ha:r0 + hb, :],
                            in_=vbs[cb][:, r0 + ha:r0 + hb, :])

    nc.compile()
    return nc


def _point_indices(rois):
    """Replicate the reference's f32 point snapping on host."""
    rr = (rois * np.float32(0.25)).reshape(-1, 4, 2)
    mids = (rr + np.roll(rr, -1, axis=1)) * np.float32(0.5)
    center = (((rr[:, 0] + rr[:, 1]) + rr[:, 2]) + rr[:, 3])
    center = (center * np.float32(0.25))[:, None, :]
    pts = np.concatenate([mids, center], axis=1)          # [N, 5, 2]
    xc = np.clip(np.ceil(pts[..., 0]), 2, 254).astype(np.int64).ravel()
    yc = np.clip(np.ceil(pts[..., 1]), 2, 254).astype(np.int64).ravel()
    return xc, yc


def kernel(feat: np.ndarray, rois: np.ndarray) -> np.ndarray:
    feat = np.ascontiguousarray(np.asarray(feat, dtype=np.float32))
    rois = np.ascontiguousarray(np.asarray(rois, dtype=np.float32))
    assert feat.shape == (C, H, W) and rois.shape == (N_ROIS, 8)

    if "nc" not in _prog_cache:
        _prog_cache["nc"] = _build_program()
    nc = _prog_cache["nc"]

    f16 = feat.astype(np.float16)
    in_maps = []
    for co in range(N_CORES):
        y0 = co * BAND
        fs = np.zeros((C, ROWS_IN, W), np.float16)
        a, b = max(0, y0 - 2), min(H, y0 + BAND + 2)
        fs[:, a - (y0 - 2):b - (y0 - 2), :] = f16[:, a:b, :]
        in_maps.append({"feat": fs})

    res = bass_utils.run_bass_kernel_spmd(
        nc, in_maps, core_ids=list(range(N_CORES)))

    # box[(y, x), c]: row-major global pixel id -> 256 channels
    box = np.empty((H * W, C), np.float16)
    bv = box.reshape(N_CORES, BAND * W, C)
    for co in range(N_CORES):
        o = np.asarray(res.results[co]["out"])   # [2, 128, BAND, W] fp16
        # -> [(ly, x), (cb, cl)]
        bv[co] = np.transpose(o.reshape(2, 128, BAND * W),
                              (2, 0, 1)).reshape(BAND * W, C)

    xc, yc = _point_indices(rois)
    pooled = box[yc * W + xc].astype(np.float32) * np.float32(1.0 / 16.0)
    return pooled.reshape(N_ROIS, 5 * C)
